# revision 1
# baseline (speedup 1.0000x reference)
"""Causal self-attention (B=4, T=2048, C=1024, NH=16, HS=64) on 8 trn2 cores.

Sharding: core = (batch b, head-group hg): b = core//2, hg = core%2.
Each core computes 8 heads of one batch: column-parallel W_attn (its heads'
q/k/v rows), row-parallel W_o (its heads' columns).  Host sums the two
head-group partials per batch and adds b_o.

Device algorithm (per core, all matmuls bf16 inputs / fp32 PSUM):
  qkv^T = W_local @ x^T            (transposed layout [j, t]; x^T, W^T prepped on host)
  rope via q*cosF + (P_swap @ (q*sinF_signed))   (P_swap = const permutation matmul)
  S^T[k,q] = K_rot^T.T @ Q_rot^T   (scores transposed, causal blocks only)
  E = exp(S^T/8) (ScalarE, fused 1/8 scale), staircase mask on diagonal blocks
  O^T|sums = [V|1]^T.T @ E         (fused unnormalized output + softmax denominator)
  O_norm = O^T * (1/sums)          (DVE multiply, partition-broadcast recip)
  y = O_cat^T.T @ W_o_cols^T       (accumulate over head pairs in PSUM)

Head-dim channels are reordered on the host (per head: even dims then odd
dims) so RoPE pairs live in contiguous 32-partition blocks; attention scores
are invariant to this permutation since q and k use the same order, and v/W_o
stay in natural order.
"""

from contextlib import ExitStack

import numpy as np
import ml_dtypes

import concourse.bass as bass
import concourse.mybir as mybir
import concourse.tile as tile
from concourse.bass_utils import run_bass_kernel_spmd
from concourse.masks import make_identity

B, T, C = 4, 2048, 1024
NH, HS = 16, 64
P = 128
NCORES = 8
NPAIR = 4            # head pairs per core (8 local heads)
CB = C // P          # 8 contraction blocks over C
QW = 512             # q-chunk width
NTC = T // QW        # 4 q-chunks
NKB = T // P         # 16 key blocks
F32 = mybir.dt.float32
BF16 = mybir.dt.bfloat16
NPBF = ml_dtypes.bfloat16
AF = mybir.ActivationFunctionType
ALU = mybir.AluOpType

_cache = {}


def _legalize_waits(nc, max_waits=1):
    """The walrus build here allows only one sync-wait command per
    instruction; move excess Tile-generated waits onto preceding
    single-wait NoOps on the same engine (same-engine program order
    makes this equivalent)."""
    n_id = [0]
    for fn in nc.m.functions:
        for blk in fn.blocks:
            out = []
            for inst in blk.instructions:
                si = inst.sync_info
                if si is not None and si.on_wait and len(si.on_wait) > max_waits:
                    waits = list(si.on_wait)
                    excess, keep = waits[:-max_waits], waits[-max_waits:]
                    for w in excess:
                        n_id[0] += 1
                        out.append(
                            mybir.InstNoOp(
                                name=f"waitsplit-{n_id[0]}",
                                engine=inst.engine,
                                bass_nofuse=True,
                                sync_info=mybir.SyncInfo(
                                    on_wait=[w], on_update=[]
                                ),
                            )
                        )
                    inst.sync_info = mybir.SyncInfo(
                        on_wait=keep, on_update=list(si.on_update)
                    )
                out.append(inst)
            blk.instructions = out
    return nc


def _build_nc():
    nc = bass.Bass(target_bir_lowering=True)
    xT_d = nc.dram_tensor("xT", [C, T], BF16, kind="ExternalInput")
    w_d = nc.dram_tensor("wqkvT", [C, 12 * P], BF16, kind="ExternalInput")
    b_d = nc.dram_tensor("bqkv", [12, P], F32, kind="ExternalInput")
    wo_d = nc.dram_tensor("woT", [4 * P, C], BF16, kind="ExternalInput")
    cos_d = nc.dram_tensor("cosF", [P, T], BF16, kind="ExternalInput")
    sin_d = nc.dram_tensor("sinF", [P, T], BF16, kind="ExternalInput")
    psw_d = nc.dram_tensor("psw", [P, P], BF16, kind="ExternalInput")
    band_d = nc.dram_tensor("band", [P, P], BF16, kind="ExternalInput")
    y_d = nc.dram_tensor("y", [T, C], F32, kind="ExternalOutput")

    with tile.TileContext(nc) as tc, ExitStack() as ctx:
        const = ctx.enter_context(tc.tile_pool(name="const", bufs=1))
        wpool = ctx.enter_context(tc.tile_pool(name="wpool", bufs=2))
        qkpool = ctx.enter_context(tc.tile_pool(name="qkpool", bufs=4))
        tmppool = ctx.enter_context(tc.tile_pool(name="tmppool", bufs=2))
        rotpool = ctx.enter_context(tc.tile_pool(name="rotpool", bufs=4))
        vpool = ctx.enter_context(tc.tile_pool(name="vpool", bufs=2))
        vnpool = ctx.enter_context(tc.tile_pool(name="vnpool", bufs=4))
        epool = ctx.enter_context(tc.tile_pool(name="epool", bufs=4))
        extpool = ctx.enter_context(tc.tile_pool(name="extpool", bufs=8))
        opool = ctx.enter_context(tc.tile_pool(name="opool", bufs=4))
        spool = ctx.enter_context(tc.tile_pool(name="spool", bufs=1))
        ypool = ctx.enter_context(tc.tile_pool(name="ypool", bufs=2))
        # PSUM budget (8 banks): qk 2x[P,2,QW]=4, av 2x[65,QW]=2, misc 2x[P,QW]=2
        ps_qk = ctx.enter_context(tc.tile_pool(name="ps_qk", bufs=2, space="PSUM"))
        ps_av = ctx.enter_context(tc.tile_pool(name="ps_av", bufs=2, space="PSUM"))
        ps_m = ctx.enter_context(tc.tile_pool(name="ps_m", bufs=2, space="PSUM"))

        # ---- constants / global loads ----
        xT_sb = const.tile([P, CB, T], BF16)
        nc.sync.dma_start(xT_sb[:], xT_d.rearrange("(cb p) t -> p cb t", p=P))
        cos_sb = const.tile([P, T], BF16)
        nc.sync.dma_start(cos_sb[:], cos_d[:])
        sin_sb = const.tile([P, T], BF16)
        nc.sync.dma_start(sin_sb[:], sin_d[:])
        psw_sb = const.tile([P, P], BF16)
        nc.sync.dma_start(psw_sb[:], psw_d[:])
        band_sb = const.tile([P, P], BF16)
        nc.sync.dma_start(band_sb[:], band_d[:])
        bias_sb = const.tile([P, 12], F32)
        nc.sync.dma_start(bias_sb[:], b_d.rearrange("j p -> p j"))
        wo_sb = const.tile([P, NPAIR, C], BF16)
        nc.sync.dma_start(wo_sb[:], wo_d.rearrange("(pr p) o -> p pr o", p=P))
        ident = const.tile([P, P], BF16)
        make_identity(nc, ident[:])
        ones_sb = const.tile([1, HS], F32)
        nc.gpsimd.memset(ones_sb[:], 1.0)

        ocat = [opool.tile([P, T], BF16, name=f"ocat{p}", tag="ocat")
                for p in range(NPAIR)]

        prep_out = {}

        def prep_stream(p):
            """Projection + rope + v-transpose for pair p.  Yields between
            PE-sized chunks so the driver can interleave with pair p-1's
            attention (PE executes its stream in order; interleaved emission
            fills exp-wait bubbles and keeps HAM warm)."""
            w_sb = wpool.tile([P, CB, 3 * P], BF16, tag="w", name="w_sb")
            nc.sync.dma_start(
                w_sb[:],
                w_d[:, 3 * P * p : 3 * P * (p + 1)].rearrange(
                    "(cb p) j -> p cb j", p=P
                ),
            )
            qkv_t = []
            for jb in range(3):
                if jb == 2:
                    dst = vpool.tile([P, T], BF16, tag="vT", name="vT")
                else:
                    dst = qkpool.tile([P, T], BF16, tag="qkT", name="qkT")
                bias_bc = bias_sb[:, 3 * p + jb : 3 * p + jb + 1].to_broadcast(
                    (P, QW)
                )
                for tc_i in range(NTC):
                    psum = ps_m.tile([P, QW], F32, tag="m", name="pj")
                    for cb in range(CB):
                        nc.tensor.matmul(
                            psum[:],
                            lhsT=w_sb[:, cb, jb * P : (jb + 1) * P],
                            rhs=xT_sb[:, cb, tc_i * QW : (tc_i + 1) * QW],
                            start=(cb == 0),
                            stop=(cb == CB - 1),
                        )
                    nc.vector.tensor_tensor(
                        dst[:, tc_i * QW : (tc_i + 1) * QW],
                        psum[:], bias_bc, ALU.add,
                    )
                    yield
                qkv_t.append(dst)

            rots = []
            for jb in range(2):  # rope: rot = t*cos + P_swap @ (t*sin_signed)
                src = qkv_t[jb]
                sq = tmppool.tile([P, T], BF16, tag="sq", name="sq")
                nc.vector.tensor_tensor(sq[:], src[:], sin_sb[:], ALU.mult)
                rot = rotpool.tile([P, T], BF16, tag="rot", name="rot")
                nc.vector.tensor_tensor(rot[:], src[:], cos_sb[:], ALU.mult)
                for tc_i in range(NTC):
                    psum = ps_m.tile([P, QW], F32, tag="m", name="sw")
                    nc.tensor.matmul(
                        psum[:],
                        lhsT=psw_sb[:],
                        rhs=sq[:, tc_i * QW : (tc_i + 1) * QW],
                        start=True,
                        stop=True,
                    )
                    nc.vector.tensor_tensor(
                        rot[:, tc_i * QW : (tc_i + 1) * QW],
                        rot[:, tc_i * QW : (tc_i + 1) * QW],
                        psum[:],
                        ALU.add,
                    )
                    yield
                rots.append(rot)

            vT = qkv_t[2]  # transpose v to [t, d] layout with ones column
            vones = []
            for h in range(2):
                vn = vnpool.tile([P, NKB, HS + 1], BF16, tag="vn", name="vn")
                nc.gpsimd.memset(vn[:, :, HS : HS + 1], 1.0)
                for g in range(2):
                    psum = ps_m.tile([P, QW], BF16, tag="m", name="vt")
                    for i in range(8):
                        kb = g * 8 + i
                        nc.tensor.matmul(
                            psum[:, i * HS : (i + 1) * HS],
                            lhsT=vT[h * HS : (h + 1) * HS, kb * P : (kb + 1) * P],
                            rhs=ident[h * HS : (h + 1) * HS, h * HS : (h + 1) * HS],
                            is_transpose=True,
                            start=True,
                            stop=True,
                        )
                    nc.scalar.activation(
                        vn[:, g * 8 : (g + 1) * 8, :HS],
                        psum[:].rearrange("p (i d) -> p i d", d=HS),
                        AF.Copy,
                    )
                    yield
                vones.append(vn)
            prep_out[p] = (rots[0], rots[1], vones)

        def attn_stream(p):
            """Attention for pair p, skewed so PE issues QKT(j+1) before
            AV(j) (AV waits on ACT's exp)."""
            rq, rk, vones = prep_out[p]
            sums = spool.tile([8, QW], F32, tag="sums", name="sums")
            lns = spool.tile([8, QW], F32, tag="lns", name="lns")
            recip = spool.tile([8, QW], F32, tag="recip", name="recip")
            exts = {}
            ps_o_cur = {}

            jobs = []
            for h in range(2):
                for qc in range(NTC):
                    ngrp = 2 * (qc + 1)
                    for g in range(ngrp):
                        jobs.append((h, qc, g, ngrp))

            def emit_qkt(h, qc, g, ngrp):
                ps_s = ps_qk.tile([P, 2, QW], F32, tag="qk", name="ps_s")
                for i in range(2):
                    kb = 2 * g + i
                    nc.tensor.matmul(
                        ps_s[:, i, :],
                        lhsT=rk[h * HS : (h + 1) * HS, kb * P : (kb + 1) * P],
                        rhs=rq[h * HS : (h + 1) * HS, qc * QW : (qc + 1) * QW],
                        start=True,
                        stop=True,
                    )
                es = epool.tile([P, 2, QW], BF16, tag="es", name="es")
                if g == ngrp - 1:
                    # last diagonal group (col offsets 256/384): exp only the
                    # visible columns, zero the rest, triangle-mask the band
                    for i in range(2):
                        kb = 2 * g + i
                        off = P * (kb - 4 * qc)
                        nc.gpsimd.memset(es[:, i, 0:off], 0.0)
                        nc.scalar.activation(
                            es[:, i, off:],
                            ps_s[:, i, off:],
                            AF.Exp,
                            scale=0.125,
                        )
                        nc.gpsimd.tensor_tensor(
                            es[:, i, off : off + P],
                            es[:, i, off : off + P],
                            band_sb[:],
                            ALU.mult,
                        )
                else:
                    nc.scalar.activation(
                        es[:].rearrange("p g q -> p (g q)"),
                        ps_s[:].rearrange("p g q -> p (g q)"),
                        AF.Exp,
                        scale=0.125,
                    )
                    if g == ngrp - 2:  # first diagonal group (offsets 0/128)
                        nc.gpsimd.memset(es[:, 1, 0:P], 0.0)
                        for i in range(2):
                            off = i * P
                            nc.gpsimd.tensor_tensor(
                                es[:, i, off : off + P],
                                es[:, i, off : off + P],
                                band_sb[:],
                                ALU.mult,
                            )
                return es

            def emit_av(h, qc, g, ngrp, es):
                if g == 0:
                    ps_o_cur[(h, qc)] = ps_av.tile(
                        [HS + 1, QW], F32, tag="av", name="ps_o"
                    )
                ps_o = ps_o_cur[(h, qc)]
                nkb = 2 * ngrp
                for i in range(2):
                    kb = 2 * g + i
                    nc.tensor.matmul(
                        ps_o[:],
                        lhsT=vones[h][:, kb, :],
                        rhs=es[:, i, :],
                        start=(kb == 0),
                        stop=(kb == nkb - 1),
                    )
                if g == ngrp - 1:
                    ext = extpool.tile([HS + 1, QW], F32, tag="ext", name="ext")
                    nc.vector.tensor_copy(ext[:], ps_o[:])
                    r = 4 * h + qc
                    nc.sync.dma_start(sums[r : r + 1, :], ext[HS : HS + 1, :])
                    exts[(h, qc)] = ext

            prev = None
            for job in jobs:
                es = emit_qkt(*job)
                yield  # prep work interleaves here, absorbing AV's exp-wait
                if prev is not None:
                    emit_av(*prev[0], prev[1])
                prev = (job, es)
                yield
            emit_av(*prev[0], prev[1])

            # 1/sums via ln+exp (same ACT table set as Exp; DVE recip is slow)
            nc.scalar.activation(lns[:], sums[:], AF.Ln)
            nc.scalar.activation(recip[:], lns[:], AF.Exp, scale=-1.0)
            # move recip rows to partition 0 so TensorE can outer-product them
            recip1d = spool.tile([1, 8 * QW], F32, tag="recip1d", name="recip1d")
            for r in range(8):
                nc.sync.dma_start(
                    recip1d[0:1, r * QW : (r + 1) * QW], recip[r : r + 1, :]
                )
            for h in range(2):
                for qc in range(NTC):
                    r = 4 * h + qc
                    # rb[d, q] = recip[q] broadcast via rank-1 ones^T @ recip
                    rb = ps_av.tile([HS, QW], F32, tag="av", name="rb")
                    nc.tensor.matmul(
                        rb[:],
                        lhsT=ones_sb[:],
                        rhs=recip1d[0:1, r * QW : (r + 1) * QW],
                        start=True,
                        stop=True,
                    )
                    nc.vector.tensor_tensor(
                        ocat[p][h * HS : (h + 1) * HS, qc * QW : (qc + 1) * QW],
                        exts[(h, qc)][:HS, :],
                        rb[:],
                        ALU.mult,
                    )
                    yield

        def drive(a_gen, b_gen, ratio=2):
            done_a = a_gen is None
            done_b = b_gen is None
            while not (done_a and done_b):
                if not done_a:
                    for _ in range(ratio):
                        try:
                            next(a_gen)
                        except StopIteration:
                            done_a = True
                            break
                if not done_b:
                    try:
                        next(b_gen)
                    except StopIteration:
                        done_b = True

        for _ in prep_stream(0):
            pass
        for p in range(NPAIR):
            drive(
                attn_stream(p),
                prep_stream(p + 1) if p + 1 < NPAIR else None,
                ratio=3,
            )

        # ---- output projection: y[t, :] = sum_p ocat_p^T @ woT_p ----
        for tb in range(NKB):
            for oc in range(2):
                psum = ps_m.tile([P, QW], F32, tag="m")
                for p in range(NPAIR):
                    nc.tensor.matmul(
                        psum[:],
                        lhsT=ocat[p][:, tb * P : (tb + 1) * P],
                        rhs=wo_sb[:, p, oc * QW : (oc + 1) * QW],
                        start=(p == 0),
                        stop=(p == NPAIR - 1),
                    )
                yb = ypool.tile([P, QW], F32, tag="yb")
                nc.vector.tensor_copy(yb[:], psum[:])
                nc.sync.dma_start(
                    y_d[tb * P : (tb + 1) * P, oc * QW : (oc + 1) * QW], yb[:]
                )
    return _legalize_waits(nc)


def _rope_tables():
    inv = 1.0 / (1000.0 ** (np.arange(0, HS, 2, dtype=np.float64) / HS))
    t = np.arange(T, dtype=np.float64)[:, None] * inv[None, :]
    sinT = np.sin(t).astype(np.float32).T  # [32, T]
    cosT = np.cos(t).astype(np.float32).T
    cosF = np.concatenate([cosT] * 4, 0)  # [128, T]
    # sign layout for multiply-BEFORE-swap: sq = q*sinF, swapped(sq) lands as
    # [-v*sin; +u*sin] in the [u; v] destination slots.
    sinF = np.concatenate([sinT, -sinT, sinT, -sinT], 0)
    return cosF, sinF


def _host_prep():
    cosF, sinF = _rope_tables()
    psw = np.zeros((P, P), np.float32)
    for hh in range(2):
        o = hh * HS
        psw[o : o + 32, o + 32 : o + 64] = np.eye(32)
        psw[o + 32 : o + 64, o : o + 32] = np.eye(32)
    # band[p, j] = 1 iff j >= p: causal triangle for the 128-wide diagonal band
    band = np.tril(np.ones((P, P), np.float32)).T
    return cosF, sinF, psw, band


def kernel(x, W_attn, b_attn, W_o, b_o, _trace=False, _tmpdir=None):
    x = np.asarray(x, np.float32)
    W_attn = np.asarray(W_attn, np.float32)
    b_attn = np.asarray(b_attn, np.float32)
    W_o = np.asarray(W_o, np.float32)
    b_o = np.asarray(b_o, np.float32)

    if "nc" not in _cache:
        _cache["nc"] = _build_nc()
    nc = _cache["nc"]

    cosF, sinF, psw, band = _host_prep()
    cosF_b, sinF_b = cosF.astype(NPBF), sinF.astype(NPBF)
    psw_b, band_b = psw.astype(NPBF), band.astype(NPBF)

    def head_rows(h):  # q-rows of head h, evens then odds
        base = h * HS
        return np.concatenate(
            [np.arange(base, base + HS, 2), np.arange(base + 1, base + HS, 2)]
        )

    in_maps = []
    for core in range(NCORES):
        b, hg = core // 2, core % 2
        heads = [hg * 8 + i for i in range(8)]
        rows = []
        for p in range(NPAIR):
            h0, h1 = heads[2 * p], heads[2 * p + 1]
            qrows = np.concatenate([head_rows(h0), head_rows(h1)])
            rows += [qrows, C + qrows,
                     2 * C + np.concatenate([np.arange(h0 * HS, (h0 + 1) * HS),
                                             np.arange(h1 * HS, (h1 + 1) * HS)])]
        rows = np.concatenate(rows)  # [1536] in pair-major (q,k,v) order
        wqkvT = np.ascontiguousarray(W_attn[rows].T).astype(NPBF)  # [C, 1536]
        bqkv = np.ascontiguousarray(b_attn[rows].reshape(12, P))
        woT = np.ascontiguousarray(
            W_o[:, hg * 512 : (hg + 1) * 512].T
        ).astype(NPBF)  # [512, C]
        xT = np.ascontiguousarray(x[b].T).astype(NPBF)  # [C, T]
        in_maps.append(
            dict(xT=xT, wqkvT=wqkvT, bqkv=bqkv, woT=woT, cosF=cosF_b,
                 sinF=sinF_b, psw=psw_b, band=band_b)
        )

    res = run_bass_kernel_spmd(nc, in_maps, core_ids=list(range(NCORES)),
                               trace=_trace, tmpdir=_tmpdir)
    y = np.zeros((B, T, C), np.float32)
    for core in range(NCORES):
        y[core // 2] += res.results[core]["y"]
    y += b_o[None, None, :]
    if _trace:
        _cache["last_result"] = res
    return y



# revision 5
# speedup vs baseline: 1.0230x; 1.0230x over previous
"""Causal self-attention (B=4, T=2048, C=1024, NH=16, HS=64) on 8 trn2 cores.

Sharding: core = (batch b, head-group hg): b = core//2, hg = core%2.
Each core computes 8 heads of one batch: column-parallel W_attn (its heads'
q/k/v rows), row-parallel W_o (its heads' columns).  Host sums the two
head-group partials per batch and adds b_o.

Device algorithm (per core, all matmuls bf16 inputs / fp32 PSUM):
  qkv^T = W_local @ x^T            (transposed layout [j, t]; x^T, W^T prepped on host)
  rope via q*cosF + (P_swap @ (q*sinF_signed))   (P_swap = const permutation matmul)
  S^T[k,q] = K_rot^T.T @ Q_rot^T   (scores transposed, causal blocks only)
  E = exp(S^T/8) (ScalarE, fused 1/8 scale), staircase mask on diagonal blocks
  O^T|sums = [V|1]^T.T @ E         (fused unnormalized output + softmax denominator)
  O_norm = O^T * (1/sums)          (DVE recip, TensorE rank-1 partition-broadcast)
  y = O_cat^T.T @ W_o_cols^T       (accumulate over head pairs in PSUM)

Perf structure (v2):
  - The two heads of a pair live in partition halves 0-63 / 64-127, so their
    K^T.T@Q^T score matmuls (contraction = HS = 64) land on different PE row
    tiles (tile_position (0,0) / (64,0)) and execute concurrently when issued
    back-to-back -> jobs are packed per (qc, kb) covering BOTH heads, with a
    single [128, 2*512] fused exp per job.
  - Softmax denominators stay local: DVE reciprocal of the fused sums row
    (partition 64) -> bf16 -> rank-1 TensorE broadcast.  No DMA round trip.
  - Out-projection for a query chunk is emitted as soon as the last pair's
    ocat columns for that chunk are normalized, so it fills pair-3 attention
    bubbles instead of serializing at the end.
  - x^T is DMA'd in per-q-chunk slices and pair-0 weights are loaded first so
    the PE starts within a few microseconds.

Head-dim channels are reordered on the host (per head: even dims then odd
dims) so RoPE pairs live in contiguous 32-partition blocks; attention scores
are invariant to this permutation since q and k use the same order, and v/W_o
stay in natural order.
"""

from contextlib import ExitStack

import numpy as np
import ml_dtypes

import concourse.bass as bass
import concourse.mybir as mybir
import concourse.tile as tile
from concourse.bass_utils import run_bass_kernel_spmd
from concourse.masks import make_identity

B, T, C = 4, 2048, 1024
NH, HS = 16, 64
P = 128
NCORES = 8
NPAIR = 4            # head pairs per core (8 local heads)
CB = C // P          # 8 contraction blocks over C
QW = 512             # q-chunk width
NTC = T // QW        # 4 q-chunks
NKB = T // P         # 16 key blocks
F32 = mybir.dt.float32
BF16 = mybir.dt.bfloat16
NPBF = ml_dtypes.bfloat16
AF = mybir.ActivationFunctionType
ALU = mybir.AluOpType

_cache = {}


def _legalize_waits(nc, max_waits=1):
    """The walrus build here allows only one sync-wait command per
    instruction; move excess Tile-generated waits onto preceding
    single-wait NoOps on the same engine (same-engine program order
    makes this equivalent)."""
    n_id = [0]
    for fn in nc.m.functions:
        for blk in fn.blocks:
            out = []
            for inst in blk.instructions:
                si = inst.sync_info
                if si is not None and si.on_wait and len(si.on_wait) > max_waits:
                    waits = list(si.on_wait)
                    excess, keep = waits[:-max_waits], waits[-max_waits:]
                    for w in excess:
                        n_id[0] += 1
                        out.append(
                            mybir.InstNoOp(
                                name=f"waitsplit-{n_id[0]}",
                                engine=inst.engine,
                                bass_nofuse=True,
                                sync_info=mybir.SyncInfo(
                                    on_wait=[w], on_update=[]
                                ),
                            )
                        )
                    inst.sync_info = mybir.SyncInfo(
                        on_wait=keep, on_update=list(si.on_update)
                    )
                out.append(inst)
            blk.instructions = out
    return nc


def _build_nc():
    nc = bass.Bass(target_bir_lowering=True)
    xT_d = nc.dram_tensor("xT", [C, T], BF16, kind="ExternalInput")
    w_d = nc.dram_tensor("wqkvT", [C, 12 * P], BF16, kind="ExternalInput")
    b_d = nc.dram_tensor("bqkv", [12, P], F32, kind="ExternalInput")
    wo_d = nc.dram_tensor("woT", [4 * P, C], BF16, kind="ExternalInput")
    cos_d = nc.dram_tensor("cosF", [P, T], BF16, kind="ExternalInput")
    sin_d = nc.dram_tensor("sinF", [P, T], BF16, kind="ExternalInput")
    psw_d = nc.dram_tensor("psw", [P, P], BF16, kind="ExternalInput")
    band_d = nc.dram_tensor("band", [P, 2 * P], BF16, kind="ExternalInput")
    y_d = nc.dram_tensor("y", [T, C], F32, kind="ExternalOutput")

    with tile.TileContext(nc) as tc, ExitStack() as ctx:
        const = ctx.enter_context(tc.tile_pool(name="const", bufs=1))
        wpool = ctx.enter_context(tc.tile_pool(name="wpool", bufs=2))
        qkpool = ctx.enter_context(tc.tile_pool(name="qkpool", bufs=4))
        tmppool = ctx.enter_context(tc.tile_pool(name="tmppool", bufs=2))
        rotpool = ctx.enter_context(tc.tile_pool(name="rotpool", bufs=4))
        vpool = ctx.enter_context(tc.tile_pool(name="vpool", bufs=2))
        vnpool = ctx.enter_context(tc.tile_pool(name="vnpool", bufs=4))
        epool = ctx.enter_context(tc.tile_pool(name="epool", bufs=4))
        extpool = ctx.enter_context(tc.tile_pool(name="extpool", bufs=3))
        rcppool = ctx.enter_context(tc.tile_pool(name="rcppool", bufs=2))
        opool = ctx.enter_context(tc.tile_pool(name="opool", bufs=4))
        ypool = ctx.enter_context(tc.tile_pool(name="ypool", bufs=2))
        # PSUM budget (8 banks): qk 2x[P,2,QW]=4, av 2x[65,QW]=2, misc 2x[P,QW]=2
        ps_qk = ctx.enter_context(tc.tile_pool(name="ps_qk", bufs=2, space="PSUM"))
        ps_av = ctx.enter_context(tc.tile_pool(name="ps_av", bufs=2, space="PSUM"))
        ps_m = ctx.enter_context(tc.tile_pool(name="ps_m", bufs=2, space="PSUM"))

        # ---- loads, ordered so pair-0 compute can start early ----
        w0_sb = wpool.tile([P, CB, 3 * P], BF16, tag="w", name="w_sb")
        nc.sync.dma_start(
            w0_sb[:], w_d[:, 0 : 3 * P].rearrange("(cb p) j -> p cb j", p=P)
        )
        xT_sb = []
        for tc_i in range(NTC):
            xt = const.tile([P, CB, QW], BF16, name=f"xT{tc_i}", tag=f"xT{tc_i}")
            nc.sync.dma_start(
                xt[:],
                xT_d[:, tc_i * QW : (tc_i + 1) * QW].rearrange(
                    "(cb p) t -> p cb t", p=P
                ),
            )
            xT_sb.append(xt)
        bias_sb = const.tile([P, 12], F32)
        nc.sync.dma_start(bias_sb[:], b_d.rearrange("j p -> p j"))
        cos_sb = const.tile([P, T], BF16)
        nc.sync.dma_start(cos_sb[:], cos_d[:])
        sin_sb = const.tile([P, T], BF16)
        nc.sync.dma_start(sin_sb[:], sin_d[:])
        psw_sb = const.tile([P, P], BF16)
        nc.sync.dma_start(psw_sb[:], psw_d[:])
        band_sb = const.tile([P, 2, P], BF16)
        nc.sync.dma_start(band_sb[:], band_d.rearrange("p (g q) -> p g q", g=2))
        ident = const.tile([P, P], BF16)
        make_identity(nc, ident[:])
        ones64 = const.tile([65, HS], BF16)
        nc.gpsimd.memset(ones64[64:65, :], 1.0)
        wo_sb = const.tile([P, NPAIR, C], BF16)
        nc.sync.dma_start(wo_sb[:], wo_d.rearrange("(pr p) o -> p pr o", p=P))

        ocat = [opool.tile([P, T], BF16, name=f"ocat{p}", tag="ocat")
                for p in range(NPAIR)]

        prep_out = {}

        def prep_stream(p, w_sb=None):
            """Projection + rope + v-transpose for pair p.  Yields between
            PE-sized chunks; emission order sets scheduler priority so this
            work fills pair p-1's attention exp-wait bubbles."""
            if w_sb is None:
                w_sb = wpool.tile([P, CB, 3 * P], BF16, tag="w", name="w_sb")
                nc.sync.dma_start(
                    w_sb[:],
                    w_d[:, 3 * P * p : 3 * P * (p + 1)].rearrange(
                        "(cb p) j -> p cb j", p=P
                    ),
                )
            qkv_t = []
            for jb in range(3):
                if jb == 2:
                    dst = vpool.tile([P, T], BF16, tag="vT", name="vT")
                else:
                    dst = qkpool.tile([P, T], BF16, tag="qkT", name="qkT")
                bias_bc = bias_sb[:, 3 * p + jb : 3 * p + jb + 1].to_broadcast(
                    (P, QW)
                )
                for tc_i in range(NTC):
                    psum = ps_m.tile([P, QW], F32, tag="m", name="pj")
                    for cb in range(CB):
                        nc.tensor.matmul(
                            psum[:],
                            lhsT=w_sb[:, cb, jb * P : (jb + 1) * P],
                            rhs=xT_sb[tc_i][:, cb, :],
                            start=(cb == 0),
                            stop=(cb == CB - 1),
                        )
                    nc.vector.tensor_tensor(
                        dst[:, tc_i * QW : (tc_i + 1) * QW],
                        psum[:], bias_bc, ALU.add,
                    )
                    yield
                qkv_t.append(dst)

            rots = []
            for jb in range(2):  # rope: rot = t*cos + P_swap @ (t*sin_signed)
                src = qkv_t[jb]
                sq = tmppool.tile([P, T], BF16, tag="sq", name="sq")
                nc.vector.tensor_tensor(sq[:], src[:], sin_sb[:], ALU.mult)
                rot = rotpool.tile([P, T], BF16, tag="rot", name="rot")
                nc.vector.tensor_tensor(rot[:], src[:], cos_sb[:], ALU.mult)
                for tc_i in range(NTC):
                    psum = ps_m.tile([P, QW], F32, tag="m", name="sw")
                    nc.tensor.matmul(
                        psum[:],
                        lhsT=psw_sb[:],
                        rhs=sq[:, tc_i * QW : (tc_i + 1) * QW],
                        start=True,
                        stop=True,
                    )
                    nc.vector.tensor_tensor(
                        rot[:, tc_i * QW : (tc_i + 1) * QW],
                        rot[:, tc_i * QW : (tc_i + 1) * QW],
                        psum[:],
                        ALU.add,
                    )
                    yield
                rots.append(rot)

            vT = qkv_t[2]  # transpose v to [t, d] layout with ones column
            vones = []
            for h in range(2):
                vn = vnpool.tile([P, NKB, HS + 1], BF16, tag="vn", name="vn")
                nc.gpsimd.memset(vn[:, :, HS : HS + 1], 1.0)
                for g in range(2):
                    psum = ps_m.tile([P, QW], BF16, tag="m", name="vt")
                    for i in range(8):
                        kb = g * 8 + i
                        nc.tensor.matmul(
                            psum[:, i * HS : (i + 1) * HS],
                            lhsT=vT[h * HS : (h + 1) * HS, kb * P : (kb + 1) * P],
                            rhs=ident[h * HS : (h + 1) * HS, h * HS : (h + 1) * HS],
                            is_transpose=True,
                            start=True,
                            stop=True,
                        )
                    nc.scalar.activation(
                        vn[:, g * 8 : (g + 1) * 8, :HS],
                        psum[:].rearrange("p (i d) -> p i d", d=HS),
                        AF.Copy,
                    )
                    yield
                vones.append(vn)
            prep_out[p] = (rots[0], rots[1], vones)

        def emit_outproj_block(tb):
            """y[tb*P:(tb+1)*P, :] = sum_p ocat_p^T @ woT_p for one 128-row
            block; emitted as soon as all pairs' ocat columns are final."""
            for oc in range(2):
                psum = ps_m.tile([P, QW], F32, tag="m", name="yp")
                for p in range(NPAIR):
                    nc.tensor.matmul(
                        psum[:],
                        lhsT=ocat[p][:, tb * P : (tb + 1) * P],
                        rhs=wo_sb[:, p, oc * QW : (oc + 1) * QW],
                        start=(p == 0),
                        stop=(p == NPAIR - 1),
                    )
                yb = ypool.tile([P, QW], F32, tag="yb")
                nc.vector.tensor_copy(yb[:], psum[:])
                nc.sync.dma_start(
                    y_d[tb * P : (tb + 1) * P, oc * QW : (oc + 1) * QW], yb[:]
                )

        def attn_stream(p):
            """Attention for pair p.  Jobs are packed per (qc, kb): both
            heads' QK^T matmuls are emitted back-to-back (concurrent PE row
            tiles 0-63 / 64-127), followed by one fused exp over both heads'
            PSUM banks and the two AV accumulations."""
            rq, rk, vones = prep_out[p]
            ps_o_cur = {}

            def normalize(ext2, qc):
                # 1/sums for both heads: ln+exp on the partition-64 sums row
                # (same ACT table as the scores exp -> no table reload)
                rcpf = rcppool.tile([65, 2, QW], F32, tag="rcpf", name="rcpf")
                nc.scalar.activation(
                    rcpf[64:65, :, :], ext2[64:65, :, :], AF.Ln
                )
                rcpb = rcppool.tile([65, 2, QW], BF16, tag="rcpb", name="rcpb")
                nc.scalar.activation(
                    rcpb[64:65, :, :], rcpf[64:65, :, :], AF.Exp, scale=-1.0
                )
                for h in range(2):
                    rb = ps_m.tile([HS, QW], F32, tag="m", name="rb")
                    nc.tensor.matmul(
                        rb[:],
                        lhsT=ones64[64:65, :],
                        rhs=rcpb[64:65, h, :],
                        start=True,
                        stop=True,
                    )
                    nc.vector.tensor_tensor(
                        ocat[p][h * HS : (h + 1) * HS,
                                qc * QW : (qc + 1) * QW],
                        ext2[:HS, h, :],
                        rb[:],
                        ALU.mult,
                    )

            for qc in range(NTC):
                nkb = 4 * (qc + 1)
                for h in range(2):
                    ps_o_cur[h] = ps_av.tile([HS + 1, QW], F32, tag="av",
                                             name="ps_o")
                for kb in range(nkb):
                    ps_s = ps_qk.tile([P, 2, QW], F32, tag="qk", name="ps_s")
                    for h in range(2):
                        nc.tensor.matmul(
                            ps_s[:, h, :],
                            lhsT=rk[h * HS : (h + 1) * HS,
                                    kb * P : (kb + 1) * P],
                            rhs=rq[h * HS : (h + 1) * HS,
                                   qc * QW : (qc + 1) * QW],
                            start=True,
                            stop=True,
                        )
                    es = epool.tile([P, 2, QW], BF16, tag="es", name="es")
                    off = P * (kb - 4 * qc)
                    if off < 0:
                        nc.scalar.activation(
                            es[:].rearrange("p g q -> p (g q)"),
                            ps_s[:].rearrange("p g q -> p (g q)"),
                            AF.Exp,
                            scale=0.125,
                        )
                    else:  # diagonal block: partial exp + staircase mask
                        if off > 0:
                            nc.gpsimd.memset(es[:, :, 0:off], 0.0)
                        nc.scalar.activation(
                            es[:, :, off:], ps_s[:, :, off:],
                            AF.Exp, scale=0.125,
                        )
                        nc.gpsimd.tensor_tensor(
                            es[:, :, off : off + P],
                            es[:, :, off : off + P],
                            band_sb[:],
                            ALU.mult,
                        )
                    for h in range(2):
                        nc.tensor.matmul(
                            ps_o_cur[h][:],
                            lhsT=vones[h][:, kb, :],
                            rhs=es[:, h, :],
                            start=(kb == 0),
                            stop=(kb == nkb - 1),
                        )
                    yield
                ext2 = extpool.tile([HS + 1, 2, QW], F32, tag="ext",
                                    name="ext2")
                for h in range(2):
                    nc.vector.tensor_copy(ext2[:, h, :], ps_o_cur[h][:])
                    yield
                normalize(ext2, qc)
                if p == NPAIR - 1:
                    for tb in range(4 * qc, 4 * qc + 4):
                        emit_outproj_block(tb)
                        yield

        def drive(a_gen, b_gen, ratio=2):
            done_a = a_gen is None
            done_b = b_gen is None
            while not (done_a and done_b):
                if not done_a:
                    for _ in range(ratio):
                        try:
                            next(a_gen)
                        except StopIteration:
                            done_a = True
                            break
                if not done_b:
                    try:
                        next(b_gen)
                    except StopIteration:
                        done_b = True

        for _ in prep_stream(0, w_sb=w0_sb):
            pass
        for p in range(NPAIR):
            drive(
                attn_stream(p),
                prep_stream(p + 1) if p + 1 < NPAIR else None,
                ratio=3,
            )
    return _legalize_waits(nc)


def _rope_tables():
    inv = 1.0 / (1000.0 ** (np.arange(0, HS, 2, dtype=np.float64) / HS))
    t = np.arange(T, dtype=np.float64)[:, None] * inv[None, :]
    sinT = np.sin(t).astype(np.float32).T  # [32, T]
    cosT = np.cos(t).astype(np.float32).T
    cosF = np.concatenate([cosT] * 4, 0)  # [128, T]
    # sign layout for multiply-BEFORE-swap: sq = q*sinF, swapped(sq) lands as
    # [-v*sin; +u*sin] in the [u; v] destination slots.
    sinF = np.concatenate([sinT, -sinT, sinT, -sinT], 0)
    return cosF, sinF


def _host_prep():
    cosF, sinF = _rope_tables()
    psw = np.zeros((P, P), np.float32)
    for hh in range(2):
        o = hh * HS
        psw[o : o + 32, o + 32 : o + 64] = np.eye(32)
        psw[o + 32 : o + 64, o : o + 32] = np.eye(32)
    # band[p, j] = 1 iff j >= p: causal triangle for the 128-wide diagonal
    # band, replicated for the two packed heads
    tri = np.tril(np.ones((P, P), np.float32)).T
    band = np.concatenate([tri, tri], axis=1)  # [P, 2P]
    return cosF, sinF, psw, band


def kernel(x, W_attn, b_attn, W_o, b_o, _trace=False, _tmpdir=None):
    x = np.asarray(x, np.float32)
    W_attn = np.asarray(W_attn, np.float32)
    b_attn = np.asarray(b_attn, np.float32)
    W_o = np.asarray(W_o, np.float32)
    b_o = np.asarray(b_o, np.float32)

    if "nc" not in _cache:
        _cache["nc"] = _build_nc()
    nc = _cache["nc"]

    cosF, sinF, psw, band = _host_prep()
    cosF_b, sinF_b = cosF.astype(NPBF), sinF.astype(NPBF)
    psw_b, band_b = psw.astype(NPBF), band.astype(NPBF)

    def head_rows(h):  # q-rows of head h, evens then odds
        base = h * HS
        return np.concatenate(
            [np.arange(base, base + HS, 2), np.arange(base + 1, base + HS, 2)]
        )

    in_maps = []
    for core in range(NCORES):
        b, hg = core // 2, core % 2
        heads = [hg * 8 + i for i in range(8)]
        rows = []
        for p in range(NPAIR):
            h0, h1 = heads[2 * p], heads[2 * p + 1]
            qrows = np.concatenate([head_rows(h0), head_rows(h1)])
            rows += [qrows, C + qrows,
                     2 * C + np.concatenate([np.arange(h0 * HS, (h0 + 1) * HS),
                                             np.arange(h1 * HS, (h1 + 1) * HS)])]
        rows = np.concatenate(rows)  # [1536] in pair-major (q,k,v) order
        wqkvT = np.ascontiguousarray(W_attn[rows].T).astype(NPBF)  # [C, 1536]
        bqkv = np.ascontiguousarray(b_attn[rows].reshape(12, P))
        woT = np.ascontiguousarray(
            W_o[:, hg * 512 : (hg + 1) * 512].T
        ).astype(NPBF)  # [512, C]
        xT = np.ascontiguousarray(x[b].T).astype(NPBF)  # [C, T]
        in_maps.append(
            dict(xT=xT, wqkvT=wqkvT, bqkv=bqkv, woT=woT, cosF=cosF_b,
                 sinF=sinF_b, psw=psw_b, band=band_b)
        )

    res = run_bass_kernel_spmd(nc, in_maps, core_ids=list(range(NCORES)),
                               trace=_trace, tmpdir=_tmpdir)
    y = np.zeros((B, T, C), np.float32)
    for core in range(NCORES):
        y[core // 2] += res.results[core]["y"]
    y += b_o[None, None, :]
    if _trace:
        _cache["last_result"] = res
    return y


# revision 9
# speedup vs baseline: 1.2429x; 1.2149x over previous
"""Causal self-attention (B=4, T=2048, C=1024, NH=16, HS=64) on 8 trn2 cores.

Sharding: core = (batch b, head-group hg): b = core//2, hg = core%2.
Each core computes 8 heads of one batch: column-parallel W_attn (its heads'
q/k/v rows), row-parallel W_o (its heads' columns).  Host sums the two
head-group partials per batch and adds b_o.

Device algorithm (per core, all matmuls bf16 inputs / fp32 PSUM):
  qkv^T = W_local @ x^T            (transposed layout [j, t]; x^T, W^T prepped on host)
  rope via q*cosF + (P_swap @ (q*sinF_signed))   (P_swap = const permutation matmul)
  S^T[k,q] = K_rot^T.T @ Q_rot^T   (scores transposed, causal blocks only)
  E = exp(S^T/8) (ScalarE, fused 1/8 scale), staircase mask on diagonal blocks
  O^T|sums = [V|1]^T.T @ E         (fused unnormalized output + softmax denominator)
  O_norm = O^T * (1/sums)          (DVE recip, TensorE rank-1 partition-broadcast)
  y = O_cat^T.T @ W_o_cols^T       (accumulate over head pairs in PSUM)

Perf structure (v2):
  - The two heads of a pair live in partition halves 0-63 / 64-127, so their
    K^T.T@Q^T score matmuls (contraction = HS = 64) land on different PE row
    tiles (tile_position (0,0) / (64,0)) and execute concurrently when issued
    back-to-back -> jobs are packed per (qc, kb) covering BOTH heads, with a
    single [128, 2*512] fused exp per job.
  - Softmax denominators stay local: DVE reciprocal of the fused sums row
    (partition 64) -> bf16 -> rank-1 TensorE broadcast.  No DMA round trip.
  - Out-projection for a query chunk is emitted as soon as the last pair's
    ocat columns for that chunk are normalized, so it fills pair-3 attention
    bubbles instead of serializing at the end.
  - x^T is DMA'd in per-q-chunk slices and pair-0 weights are loaded first so
    the PE starts within a few microseconds.

Head-dim channels are reordered on the host (per head: even dims then odd
dims) so RoPE pairs live in contiguous 32-partition blocks; attention scores
are invariant to this permutation since q and k use the same order, and v/W_o
stay in natural order.
"""

from contextlib import ExitStack

import numpy as np
import ml_dtypes

import concourse.bass as bass
import concourse.mybir as mybir
import concourse.tile as tile
from concourse.bass_utils import run_bass_kernel_spmd
from concourse.masks import make_identity

B, T, C = 4, 2048, 1024
NH, HS = 16, 64
P = 128
NCORES = 8
NPAIR = 4            # head pairs per core (8 local heads)
CB = C // P          # 8 contraction blocks over C
QW = 512             # q-chunk width
NTC = T // QW        # 4 q-chunks
NKB = T // P         # 16 key blocks
F32 = mybir.dt.float32
BF16 = mybir.dt.bfloat16
NPBF = ml_dtypes.bfloat16
AF = mybir.ActivationFunctionType
ALU = mybir.AluOpType

_cache = {}


def _legalize_waits(nc, max_waits=1):
    """The walrus build here allows only one sync-wait command per
    instruction; move excess Tile-generated waits onto preceding
    single-wait NoOps on the same engine (same-engine program order
    makes this equivalent)."""
    n_id = [0]
    for fn in nc.m.functions:
        for blk in fn.blocks:
            out = []
            for inst in blk.instructions:
                si = inst.sync_info
                if si is not None and si.on_wait and len(si.on_wait) > max_waits:
                    waits = list(si.on_wait)
                    excess, keep = waits[:-max_waits], waits[-max_waits:]
                    for w in excess:
                        n_id[0] += 1
                        out.append(
                            mybir.InstNoOp(
                                name=f"waitsplit-{n_id[0]}",
                                engine=inst.engine,
                                bass_nofuse=True,
                                sync_info=mybir.SyncInfo(
                                    on_wait=[w], on_update=[]
                                ),
                            )
                        )
                    inst.sync_info = mybir.SyncInfo(
                        on_wait=keep, on_update=list(si.on_update)
                    )
                out.append(inst)
            blk.instructions = out
    return nc


def _build_nc():
    nc = bass.Bass(target_bir_lowering=True)
    xT_d = nc.dram_tensor("xT", [C, T], BF16, kind="ExternalInput")
    w_d = nc.dram_tensor("wqkvT", [C, 12 * P], BF16, kind="ExternalInput")
    b_d = nc.dram_tensor("bqkv", [12, P], F32, kind="ExternalInput")
    wo_d = nc.dram_tensor("woT", [4 * P, C], BF16, kind="ExternalInput")
    cos_d = nc.dram_tensor("cosF", [P, T], BF16, kind="ExternalInput")
    sin_d = nc.dram_tensor("sinF", [P, T], BF16, kind="ExternalInput")
    psw_d = nc.dram_tensor("psw", [P, P], BF16, kind="ExternalInput")
    band_d = nc.dram_tensor("band", [P, 2 * P], BF16, kind="ExternalInput")
    y_d = nc.dram_tensor("y", [T, C], F32, kind="ExternalOutput")

    with tile.TileContext(nc) as tc, ExitStack() as ctx:
        const = ctx.enter_context(tc.tile_pool(name="const", bufs=1))
        wpool = ctx.enter_context(tc.tile_pool(name="wpool", bufs=2))
        qkpool = ctx.enter_context(tc.tile_pool(name="qkpool", bufs=4))
        tmppool = ctx.enter_context(tc.tile_pool(name="tmppool", bufs=2))
        rotpool = ctx.enter_context(tc.tile_pool(name="rotpool", bufs=4))
        vpool = ctx.enter_context(tc.tile_pool(name="vpool", bufs=2))
        vnpool = ctx.enter_context(tc.tile_pool(name="vnpool", bufs=4))
        epool = ctx.enter_context(tc.tile_pool(name="epool", bufs=6))
        extpool = ctx.enter_context(tc.tile_pool(name="extpool", bufs=3))
        rcppool = ctx.enter_context(tc.tile_pool(name="rcppool", bufs=2))
        opool = ctx.enter_context(tc.tile_pool(name="opool", bufs=4))
        ypool = ctx.enter_context(tc.tile_pool(name="ypool", bufs=2))
        # PSUM budget (8 banks): qk 2x[P,2,QW]=4, av 2x[65,QW]=2, misc 2x[P,QW]=2
        ps_qk = ctx.enter_context(tc.tile_pool(name="ps_qk", bufs=2, space="PSUM"))
        ps_av = ctx.enter_context(tc.tile_pool(name="ps_av", bufs=2, space="PSUM"))
        ps_m = ctx.enter_context(tc.tile_pool(name="ps_m", bufs=2, space="PSUM"))

        # ---- loads, ordered so pair-0 compute can start early ----
        w0_sb = wpool.tile([P, CB, 3 * P], BF16, tag="w", name="w_sb")
        nc.sync.dma_start(
            w0_sb[:], w_d[:, 0 : 3 * P].rearrange("(cb p) j -> p cb j", p=P)
        )
        xT_sb = []
        for tc_i in range(NTC):
            xt = const.tile([P, CB, QW], BF16, name=f"xT{tc_i}", tag=f"xT{tc_i}")
            nc.sync.dma_start(
                xt[:],
                xT_d[:, tc_i * QW : (tc_i + 1) * QW].rearrange(
                    "(cb p) t -> p cb t", p=P
                ),
            )
            xT_sb.append(xt)
        bias_sb = const.tile([P, 12], F32)
        nc.sync.dma_start(bias_sb[:], b_d.rearrange("j p -> p j"))
        cos_sb = const.tile([P, T], BF16)
        nc.sync.dma_start(cos_sb[:], cos_d[:])
        sin_sb = const.tile([P, T], BF16)
        nc.sync.dma_start(sin_sb[:], sin_d[:])
        psw_sb = const.tile([P, P], BF16)
        nc.sync.dma_start(psw_sb[:], psw_d[:])
        band_sb = const.tile([P, 2, P], BF16)
        nc.sync.dma_start(band_sb[:], band_d.rearrange("p (g q) -> p g q", g=2))
        ident = const.tile([P, P], BF16)
        make_identity(nc, ident[:])
        ones64 = const.tile([65, HS], BF16)
        nc.gpsimd.memset(ones64[64:65, :], 1.0)
        wo_sb = const.tile([P, NPAIR, C], BF16)
        nc.sync.dma_start(wo_sb[:], wo_d.rearrange("(pr p) o -> p pr o", p=P))

        ocat = [opool.tile([P, T], BF16, name=f"ocat{p}", tag="ocat")
                for p in range(NPAIR)]

        prep_out = {}

        def prep_stream(p, w_sb=None):
            """Projection + rope + v-transpose for pair p.  Yields between
            PE-sized chunks; emission order sets scheduler priority so this
            work fills pair p-1's attention exp-wait bubbles."""
            if w_sb is None:
                w_sb = wpool.tile([P, CB, 3 * P], BF16, tag="w", name="w_sb")
                nc.sync.dma_start(
                    w_sb[:],
                    w_d[:, 3 * P * p : 3 * P * (p + 1)].rearrange(
                        "(cb p) j -> p cb j", p=P
                    ),
                )
            qkv_t = []
            for jb in range(3):
                if jb == 2:
                    dst = vpool.tile([P, T], BF16, tag="vT", name="vT")
                else:
                    dst = qkpool.tile([P, T], BF16, tag="qkT", name="qkT")
                bias_bc = bias_sb[:, 3 * p + jb : 3 * p + jb + 1].to_broadcast(
                    (P, QW)
                )
                for tc_i in range(NTC):
                    psum = ps_m.tile([P, QW], F32, tag="m", name="pj")
                    for cb in range(CB):
                        nc.tensor.matmul(
                            psum[:],
                            lhsT=w_sb[:, cb, jb * P : (jb + 1) * P],
                            rhs=xT_sb[tc_i][:, cb, :],
                            start=(cb == 0),
                            stop=(cb == CB - 1),
                        )
                    nc.vector.tensor_tensor(
                        dst[:, tc_i * QW : (tc_i + 1) * QW],
                        psum[:], bias_bc, ALU.add,
                    )
                    yield
                qkv_t.append(dst)

            rots = []
            for jb in range(2):  # rope: rot = t*cos + P_swap @ (t*sin_signed)
                src = qkv_t[jb]
                sq = tmppool.tile([P, T], BF16, tag="sq", name="sq")
                rot = rotpool.tile([P, T], BF16, tag="rot", name="rot")
                for tc_i in range(NTC):  # per-chunk so tc 0 starts before xT3
                    sl = slice(tc_i * QW, (tc_i + 1) * QW)
                    nc.vector.tensor_tensor(
                        sq[:, sl], src[:, sl], sin_sb[:, sl], ALU.mult
                    )
                    nc.vector.tensor_tensor(
                        rot[:, sl], src[:, sl], cos_sb[:, sl], ALU.mult
                    )
                    psum = ps_m.tile([P, QW], F32, tag="m", name="sw")
                    nc.tensor.matmul(
                        psum[:],
                        lhsT=psw_sb[:],
                        rhs=sq[:, sl],
                        start=True,
                        stop=True,
                    )
                    nc.vector.tensor_tensor(
                        rot[:, sl], rot[:, sl], psum[:], ALU.add,
                    )
                    yield
                rots.append(rot)

            vT = qkv_t[2]  # transpose v to [t, d] layout with ones column
            vones = []
            for h in range(2):
                vn = vnpool.tile([P, NKB, HS + 1], BF16, tag="vn", name="vn")
                nc.gpsimd.memset(vn[:, :, HS : HS + 1], 1.0)
                for g in range(2):
                    psum = ps_m.tile([P, QW], BF16, tag="m", name="vt")
                    for i in range(8):
                        kb = g * 8 + i
                        nc.tensor.matmul(
                            psum[:, i * HS : (i + 1) * HS],
                            lhsT=vT[h * HS : (h + 1) * HS, kb * P : (kb + 1) * P],
                            rhs=ident[h * HS : (h + 1) * HS, h * HS : (h + 1) * HS],
                            is_transpose=True,
                            start=True,
                            stop=True,
                        )
                    nc.vector.tensor_copy(
                        vn[:, g * 8 : (g + 1) * 8, :HS],
                        psum[:].rearrange("p (i d) -> p i d", d=HS),
                    )
                    yield
                vones.append(vn)
            prep_out[p] = (rots[0], rots[1], vones)

        def emit_outproj_block(tb):
            """y[tb*P:(tb+1)*P, :] = sum_p ocat_p^T @ woT_p for one 128-row
            block; emitted as soon as all pairs' ocat columns are final."""
            for oc in range(2):
                psum = ps_m.tile([P, QW], F32, tag="m", name="yp")
                for p in range(NPAIR):
                    nc.tensor.matmul(
                        psum[:],
                        lhsT=ocat[p][:, tb * P : (tb + 1) * P],
                        rhs=wo_sb[:, p, oc * QW : (oc + 1) * QW],
                        start=(p == 0),
                        stop=(p == NPAIR - 1),
                    )
                yb = ypool.tile([P, QW], F32, tag="yb")
                nc.vector.tensor_copy(yb[:], psum[:])
                nc.sync.dma_start(
                    y_d[tb * P : (tb + 1) * P, oc * QW : (oc + 1) * QW], yb[:]
                )

        def attn_stream(p):
            """Attention for pair p.  Jobs are packed per (qc, kb): both
            heads' QK^T matmuls are emitted back-to-back (concurrent PE row
            tiles 0-63 / 64-127), followed by one fused exp over both heads'
            PSUM banks and the two AV accumulations."""
            rq, rk, vones = prep_out[p]
            ps_o_cur = {}

            def normalize(ext2, qc):
                # 1/sums for both heads: ln+exp on the partition-64 sums row
                # (same ACT table as the scores exp -> no table reload)
                rcpf = rcppool.tile([65, 2, QW], F32, tag="rcpf", name="rcpf")
                nc.scalar.activation(
                    rcpf[64:65, :, :], ext2[64:65, :, :], AF.Ln
                )
                rcpb = rcppool.tile([65, 2, QW], BF16, tag="rcpb", name="rcpb")
                nc.scalar.activation(
                    rcpb[64:65, :, :], rcpf[64:65, :, :], AF.Exp, scale=-1.0
                )
                for h in range(2):
                    rb = ps_m.tile([HS, QW], F32, tag="m", name="rb")
                    nc.tensor.matmul(
                        rb[:],
                        lhsT=ones64[64:65, :],
                        rhs=rcpb[64:65, h, :],
                        start=True,
                        stop=True,
                    )
                    nc.vector.tensor_tensor(
                        ocat[p][h * HS : (h + 1) * HS,
                                qc * QW : (qc + 1) * QW],
                        ext2[:HS, h, :],
                        rb[:],
                        ALU.mult,
                    )

            for qc in range(NTC):
                nkb = 4 * (qc + 1)
                for h in range(2):
                    ps_o_cur[h] = ps_av.tile([HS + 1, QW], F32, tag="av",
                                             name="ps_o")
                for kb in range(nkb):
                    ps_s = ps_qk.tile([P, 2, QW], F32, tag="qk", name="ps_s")
                    for h in range(2):
                        nc.tensor.matmul(
                            ps_s[:, h, :],
                            lhsT=rk[h * HS : (h + 1) * HS,
                                    kb * P : (kb + 1) * P],
                            rhs=rq[h * HS : (h + 1) * HS,
                                   qc * QW : (qc + 1) * QW],
                            start=True,
                            stop=True,
                        )
                    es = epool.tile([P, 2, QW], BF16, tag="es", name="es")
                    off = P * (kb - 4 * qc)
                    if off < 0:
                        nc.scalar.activation(
                            es[:].rearrange("p g q -> p (g q)"),
                            ps_s[:].rearrange("p g q -> p (g q)"),
                            AF.Exp,
                            scale=0.125,
                        )
                    else:  # diagonal block: partial exp + staircase mask
                        if off > 0:
                            nc.gpsimd.memset(es[:, :, 0:off], 0.0)
                        nc.scalar.activation(
                            es[:, :, off:], ps_s[:, :, off:],
                            AF.Exp, scale=0.125,
                        )
                        nc.gpsimd.tensor_tensor(
                            es[:, :, off : off + P],
                            es[:, :, off : off + P],
                            band_sb[:],
                            ALU.mult,
                        )
                    # Late priority: when several PE instructions are ready
                    # the scheduler prefers QKT/prep work, so AVs trail exp
                    # by as much as the es pool depth allows (robust to
                    # cost-model vs hardware timing skew).
                    with tc.high_priority(offset=-1_000_000):
                        for h in range(2):
                            nc.tensor.matmul(
                                ps_o_cur[h][:],
                                lhsT=vones[h][:, kb, :],
                                rhs=es[:, h, :],
                                start=(kb == 0),
                                stop=(kb == nkb - 1),
                            )
                    yield
                with tc.high_priority(offset=-1_000_000):
                    ext2 = extpool.tile([HS + 1, 2, QW], F32, tag="ext",
                                        name="ext2")
                    for h in range(2):
                        nc.vector.tensor_copy(ext2[:, h, :], ps_o_cur[h][:])
                    normalize(ext2, qc)
                yield
                if p == NPAIR - 1:
                    with tc.high_priority(offset=-1_000_000):
                        for tb in range(4 * qc, 4 * qc + 4):
                            emit_outproj_block(tb)
                    yield

        def drive(a_gen, b_gen, ratio=2):
            done_a = a_gen is None
            done_b = b_gen is None
            while not (done_a and done_b):
                if not done_a:
                    for _ in range(ratio):
                        try:
                            next(a_gen)
                        except StopIteration:
                            done_a = True
                            break
                if not done_b:
                    try:
                        next(b_gen)
                    except StopIteration:
                        done_b = True

        for _ in prep_stream(0, w_sb=w0_sb):
            pass
        for p in range(NPAIR):
            drive(
                attn_stream(p),
                prep_stream(p + 1) if p + 1 < NPAIR else None,
                ratio=3,
            )
    return _legalize_waits(nc)


def _rope_tables():
    inv = 1.0 / (1000.0 ** (np.arange(0, HS, 2, dtype=np.float64) / HS))
    t = np.arange(T, dtype=np.float64)[:, None] * inv[None, :]
    sinT = np.sin(t).astype(np.float32).T  # [32, T]
    cosT = np.cos(t).astype(np.float32).T
    cosF = np.concatenate([cosT] * 4, 0)  # [128, T]
    # sign layout for multiply-BEFORE-swap: sq = q*sinF, swapped(sq) lands as
    # [-v*sin; +u*sin] in the [u; v] destination slots.
    sinF = np.concatenate([sinT, -sinT, sinT, -sinT], 0)
    return cosF, sinF


def _host_prep():
    cosF, sinF = _rope_tables()
    psw = np.zeros((P, P), np.float32)
    for hh in range(2):
        o = hh * HS
        psw[o : o + 32, o + 32 : o + 64] = np.eye(32)
        psw[o + 32 : o + 64, o : o + 32] = np.eye(32)
    # band[p, j] = 1 iff j >= p: causal triangle for the 128-wide diagonal
    # band, replicated for the two packed heads
    tri = np.tril(np.ones((P, P), np.float32)).T
    band = np.concatenate([tri, tri], axis=1)  # [P, 2P]
    return cosF, sinF, psw, band


def kernel(x, W_attn, b_attn, W_o, b_o, _trace=False, _tmpdir=None):
    x = np.asarray(x, np.float32)
    W_attn = np.asarray(W_attn, np.float32)
    b_attn = np.asarray(b_attn, np.float32)
    W_o = np.asarray(W_o, np.float32)
    b_o = np.asarray(b_o, np.float32)

    if "nc" not in _cache:
        _cache["nc"] = _build_nc()
    nc = _cache["nc"]

    cosF, sinF, psw, band = _host_prep()
    cosF_b, sinF_b = cosF.astype(NPBF), sinF.astype(NPBF)
    psw_b, band_b = psw.astype(NPBF), band.astype(NPBF)

    def head_rows(h):  # q-rows of head h, evens then odds
        base = h * HS
        return np.concatenate(
            [np.arange(base, base + HS, 2), np.arange(base + 1, base + HS, 2)]
        )

    in_maps = []
    for core in range(NCORES):
        b, hg = core // 2, core % 2
        heads = [hg * 8 + i for i in range(8)]
        rows = []
        for p in range(NPAIR):
            h0, h1 = heads[2 * p], heads[2 * p + 1]
            qrows = np.concatenate([head_rows(h0), head_rows(h1)])
            rows += [qrows, C + qrows,
                     2 * C + np.concatenate([np.arange(h0 * HS, (h0 + 1) * HS),
                                             np.arange(h1 * HS, (h1 + 1) * HS)])]
        rows = np.concatenate(rows)  # [1536] in pair-major (q,k,v) order
        wqkvT = np.ascontiguousarray(W_attn[rows].T).astype(NPBF)  # [C, 1536]
        bqkv = np.ascontiguousarray(b_attn[rows].reshape(12, P))
        woT = np.ascontiguousarray(
            W_o[:, hg * 512 : (hg + 1) * 512].T
        ).astype(NPBF)  # [512, C]
        xT = np.ascontiguousarray(x[b].T).astype(NPBF)  # [C, T]
        in_maps.append(
            dict(xT=xT, wqkvT=wqkvT, bqkv=bqkv, woT=woT, cosF=cosF_b,
                 sinF=sinF_b, psw=psw_b, band=band_b)
        )

    res = run_bass_kernel_spmd(nc, in_maps, core_ids=list(range(NCORES)),
                               trace=_trace, tmpdir=_tmpdir)
    y = np.zeros((B, T, C), np.float32)
    for core in range(NCORES):
        y[core // 2] += res.results[core]["y"]
    y += b_o[None, None, :]
    if _trace:
        _cache["last_result"] = res
    return y


# revision 13
# speedup vs baseline: 1.2811x; 1.0308x over previous
"""Causal self-attention (B=4, T=2048, C=1024, NH=16, HS=64) on 8 trn2 cores.

Sharding: core = (batch b, head-group hg): b = core//2, hg = core%2.
Each core computes 8 heads of one batch: column-parallel W_attn (its heads'
q/k/v rows), row-parallel W_o (its heads' columns).  Host sums the two
head-group partials per batch and adds b_o.

Device algorithm (per core, all matmuls bf16 inputs / fp32 PSUM):
  qkv^T = W_local @ x^T            (transposed layout [j, t]; x^T, W^T prepped on host)
  rope via q*cosF + (P_swap @ (q*sinF_signed))   (P_swap = const permutation matmul)
  S^T[k,q] = K_rot^T.T @ Q_rot^T   (scores transposed, causal blocks only)
  E = exp(S^T/8) (ScalarE, fused 1/8 scale), staircase mask on diagonal blocks
  O^T|sums = [V|1]^T.T @ E         (fused unnormalized output + softmax denominator)
  O_norm = O^T * (1/sums)          (DVE recip, TensorE rank-1 partition-broadcast)
  y = O_cat^T.T @ W_o_cols^T       (accumulate over head pairs in PSUM)

Perf structure (v2):
  - The two heads of a pair live in partition halves 0-63 / 64-127, so their
    K^T.T@Q^T score matmuls (contraction = HS = 64) land on different PE row
    tiles (tile_position (0,0) / (64,0)) and execute concurrently when issued
    back-to-back -> jobs are packed per (qc, kb) covering BOTH heads, with a
    single [128, 2*512] fused exp per job.
  - Softmax denominators stay local: DVE reciprocal of the fused sums row
    (partition 64) -> bf16 -> rank-1 TensorE broadcast.  No DMA round trip.
  - Out-projection for a query chunk is emitted as soon as the last pair's
    ocat columns for that chunk are normalized, so it fills pair-3 attention
    bubbles instead of serializing at the end.
  - x^T is DMA'd in per-q-chunk slices and pair-0 weights are loaded first so
    the PE starts within a few microseconds.

Head-dim channels are reordered on the host (per head: even dims then odd
dims) so RoPE pairs live in contiguous 32-partition blocks; attention scores
are invariant to this permutation since q and k use the same order, and v/W_o
stay in natural order.
"""

from contextlib import ExitStack

import numpy as np
import ml_dtypes

import concourse.bass as bass
import concourse.mybir as mybir
import concourse.tile as tile
from concourse.bass_utils import run_bass_kernel_spmd
from concourse.masks import make_identity

B, T, C = 4, 2048, 1024
NH, HS = 16, 64
P = 128
NCORES = 8
NPAIR = 4            # head pairs per core (8 local heads)
CB = C // P          # 8 contraction blocks over C
QW = 512             # q-chunk width
NTC = T // QW        # 4 q-chunks
NKB = T // P         # 16 key blocks
F32 = mybir.dt.float32
BF16 = mybir.dt.bfloat16
NPBF = ml_dtypes.bfloat16
AF = mybir.ActivationFunctionType
ALU = mybir.AluOpType

_cache = {}


def _legalize_waits(nc, max_waits=1):
    """The walrus build here allows only one sync-wait command per
    instruction; move excess Tile-generated waits onto preceding
    single-wait NoOps on the same engine (same-engine program order
    makes this equivalent)."""
    n_id = [0]
    for fn in nc.m.functions:
        for blk in fn.blocks:
            out = []
            for inst in blk.instructions:
                si = inst.sync_info
                if si is not None and si.on_wait and len(si.on_wait) > max_waits:
                    waits = list(si.on_wait)
                    excess, keep = waits[:-max_waits], waits[-max_waits:]
                    for w in excess:
                        n_id[0] += 1
                        out.append(
                            mybir.InstNoOp(
                                name=f"waitsplit-{n_id[0]}",
                                engine=inst.engine,
                                bass_nofuse=True,
                                sync_info=mybir.SyncInfo(
                                    on_wait=[w], on_update=[]
                                ),
                            )
                        )
                    inst.sync_info = mybir.SyncInfo(
                        on_wait=keep, on_update=list(si.on_update)
                    )
                out.append(inst)
            blk.instructions = out
    return nc


def _build_nc():
    nc = bass.Bass(target_bir_lowering=True)
    xT_d = nc.dram_tensor("xT", [C, T], BF16, kind="ExternalInput")
    w_d = nc.dram_tensor("wqkvT", [C, 12 * P], BF16, kind="ExternalInput")
    b_d = nc.dram_tensor("bqkv", [12, P], F32, kind="ExternalInput")
    wo_d = nc.dram_tensor("woT", [4 * P, C], BF16, kind="ExternalInput")
    cos_d = nc.dram_tensor("cosF", [P, T], BF16, kind="ExternalInput")
    sin_d = nc.dram_tensor("sinF", [P, T], BF16, kind="ExternalInput")
    psw_d = nc.dram_tensor("psw", [P, P], BF16, kind="ExternalInput")
    band_d = nc.dram_tensor("band", [P, 2 * P], BF16, kind="ExternalInput")
    y_d = nc.dram_tensor("y", [T, C], F32, kind="ExternalOutput")

    with tile.TileContext(nc) as tc, ExitStack() as ctx:
        const = ctx.enter_context(tc.tile_pool(name="const", bufs=1))
        wpool = ctx.enter_context(tc.tile_pool(name="wpool", bufs=2))
        qkpool = ctx.enter_context(tc.tile_pool(name="qkpool", bufs=4))
        tmppool = ctx.enter_context(tc.tile_pool(name="tmppool", bufs=2))
        rotpool = ctx.enter_context(tc.tile_pool(name="rotpool", bufs=4))
        vpool = ctx.enter_context(tc.tile_pool(name="vpool", bufs=2))
        vnpool = ctx.enter_context(tc.tile_pool(name="vnpool", bufs=4))
        epool = ctx.enter_context(tc.tile_pool(name="epool", bufs=6))
        extpool = ctx.enter_context(tc.tile_pool(name="extpool", bufs=3))
        rcppool = ctx.enter_context(tc.tile_pool(name="rcppool", bufs=2))
        opool = ctx.enter_context(tc.tile_pool(name="opool", bufs=4))
        ypool = ctx.enter_context(tc.tile_pool(name="ypool", bufs=2))
        # PSUM budget (8 banks): qk 2x[P,2,QW]=4, av 2x[65,QW]=2, misc 2x[P,QW]=2
        ps_qk = ctx.enter_context(tc.tile_pool(name="ps_qk", bufs=2, space="PSUM"))
        ps_av = ctx.enter_context(tc.tile_pool(name="ps_av", bufs=2, space="PSUM"))
        ps_m = ctx.enter_context(tc.tile_pool(name="ps_m", bufs=2, space="PSUM"))

        # ---- loads, ordered so pair-0 compute can start early ----
        # split the big strided loads into halves: each dma_start lands on a
        # single hardware queue, so halves double the effective bandwidth
        def load_w(w_sb, p):
            for cbh in range(2):
                nc.sync.dma_start(
                    w_sb[:, 4 * cbh : 4 * cbh + 4, :],
                    w_d[4 * cbh * P : (4 * cbh + 4) * P,
                        3 * P * p : 3 * P * (p + 1)].rearrange(
                        "(cb p) j -> p cb j", p=P
                    ),
                )

        w0_sb = wpool.tile([P, CB, 3 * P], BF16, tag="w", name="w_sb")
        load_w(w0_sb, 0)
        xT_sb = []
        for tc_i in range(NTC):
            xt = const.tile([P, CB, QW], BF16, name=f"xT{tc_i}", tag=f"xT{tc_i}")
            for cbh in range(2):
                nc.sync.dma_start(
                    xt[:, 4 * cbh : 4 * cbh + 4, :],
                    xT_d[4 * cbh * P : (4 * cbh + 4) * P,
                         tc_i * QW : (tc_i + 1) * QW].rearrange(
                        "(cb p) t -> p cb t", p=P
                    ),
                )
            xT_sb.append(xt)
        bias_sb = const.tile([P, 12], F32)
        nc.sync.dma_start(bias_sb[:], b_d.rearrange("j p -> p j"))
        cos_sb = const.tile([P, T], BF16)
        nc.sync.dma_start(cos_sb[:], cos_d[:])
        sin_sb = const.tile([P, T], BF16)
        nc.sync.dma_start(sin_sb[:], sin_d[:])
        psw_sb = const.tile([P, P], BF16)
        nc.sync.dma_start(psw_sb[:], psw_d[:])
        band_sb = const.tile([P, 2, P], BF16)
        nc.sync.dma_start(band_sb[:], band_d.rearrange("p (g q) -> p g q", g=2))
        ident = const.tile([P, P], BF16)
        make_identity(nc, ident[:])
        ones64 = const.tile([65, HS], BF16)
        nc.gpsimd.memset(ones64[64:65, :], 1.0)
        wo_sb = const.tile([P, NPAIR, C], BF16)
        nc.sync.dma_start(wo_sb[:], wo_d.rearrange("(pr p) o -> p pr o", p=P))

        ocat = [opool.tile([P, T], BF16, name=f"ocat{p}", tag="ocat")
                for p in range(NPAIR)]

        prep_out = {}

        def prep_stream(p, w_sb=None):
            """Projection + rope + v-transpose for pair p.  Yields between
            PE-sized chunks; emission order sets scheduler priority so this
            work fills pair p-1's attention exp-wait bubbles."""
            if w_sb is None:
                w_sb = wpool.tile([P, CB, 3 * P], BF16, tag="w", name="w_sb")
                load_w(w_sb, p)
            qkv_t = []
            for jb in range(3):
                if jb == 2:
                    dst = vpool.tile([P, T], BF16, tag="vT", name="vT")
                else:
                    dst = qkpool.tile([P, T], BF16, tag="qkT", name="qkT")
                bias_bc = bias_sb[:, 3 * p + jb : 3 * p + jb + 1].to_broadcast(
                    (P, QW)
                )
                for tc_i in range(NTC):
                    psum = ps_m.tile([P, QW], F32, tag="m", name="pj")
                    for cb in range(CB):
                        nc.tensor.matmul(
                            psum[:],
                            lhsT=w_sb[:, cb, jb * P : (jb + 1) * P],
                            rhs=xT_sb[tc_i][:, cb, :],
                            start=(cb == 0),
                            stop=(cb == CB - 1),
                        )
                    nc.vector.tensor_tensor(
                        dst[:, tc_i * QW : (tc_i + 1) * QW],
                        psum[:], bias_bc, ALU.add,
                    )
                    yield
                qkv_t.append(dst)

            rots = []
            for jb in range(2):  # rope: rot = t*cos + P_swap @ (t*sin_signed)
                src = qkv_t[jb]
                sq = tmppool.tile([P, T], BF16, tag="sq", name="sq")
                rot = rotpool.tile([P, T], BF16, tag="rot", name="rot")
                for tc_i in range(NTC):  # per-chunk so tc 0 starts before xT3
                    sl = slice(tc_i * QW, (tc_i + 1) * QW)
                    nc.vector.tensor_tensor(
                        sq[:, sl], src[:, sl], sin_sb[:, sl], ALU.mult
                    )
                    nc.vector.tensor_tensor(
                        rot[:, sl], src[:, sl], cos_sb[:, sl], ALU.mult
                    )
                    psum = ps_m.tile([P, QW], F32, tag="m", name="sw")
                    nc.tensor.matmul(
                        psum[:],
                        lhsT=psw_sb[:],
                        rhs=sq[:, sl],
                        start=True,
                        stop=True,
                    )
                    nc.vector.tensor_tensor(
                        rot[:, sl], rot[:, sl], psum[:], ALU.add,
                    )
                    yield
                rots.append(rot)

            vT = qkv_t[2]  # transpose v to [t, d] layout with ones column
            vones = []
            for h in range(2):
                vn = vnpool.tile([P, NKB, HS + 1], BF16, tag="vn", name="vn")
                nc.gpsimd.memset(vn[:, :, HS : HS + 1], 1.0)
                for g in range(2):
                    psum = ps_m.tile([P, QW], BF16, tag="m", name="vt")
                    for i in range(8):
                        kb = g * 8 + i
                        nc.tensor.matmul(
                            psum[:, i * HS : (i + 1) * HS],
                            lhsT=vT[h * HS : (h + 1) * HS, kb * P : (kb + 1) * P],
                            rhs=ident[h * HS : (h + 1) * HS, h * HS : (h + 1) * HS],
                            is_transpose=True,
                            start=True,
                            stop=True,
                        )
                    nc.vector.tensor_copy(
                        vn[:, g * 8 : (g + 1) * 8, :HS],
                        psum[:].rearrange("p (i d) -> p i d", d=HS),
                    )
                    yield
                vones.append(vn)
            prep_out[p] = (rots[0], rots[1], vones)

        def emit_outproj_block(tb):
            """y[tb*P:(tb+1)*P, :] = sum_p ocat_p^T @ woT_p for one 128-row
            block; emitted as soon as all pairs' ocat columns are final."""
            for oc in range(2):
                psum = ps_m.tile([P, QW], F32, tag="m", name="yp")
                for p in range(NPAIR):
                    nc.tensor.matmul(
                        psum[:],
                        lhsT=ocat[p][:, tb * P : (tb + 1) * P],
                        rhs=wo_sb[:, p, oc * QW : (oc + 1) * QW],
                        start=(p == 0),
                        stop=(p == NPAIR - 1),
                    )
                yb = ypool.tile([P, QW], F32, tag="yb")
                nc.vector.tensor_copy(yb[:], psum[:])
                nc.sync.dma_start(
                    y_d[tb * P : (tb + 1) * P, oc * QW : (oc + 1) * QW], yb[:]
                )

        def attn_stream(p):
            """Attention for pair p.  Jobs are packed per (qc, kb): both
            heads' QK^T matmuls are emitted back-to-back (concurrent PE row
            tiles 0-63 / 64-127), followed by one fused exp over both heads'
            PSUM banks and the two AV accumulations."""
            rq, rk, vones = prep_out[p]
            ps_o_cur = {}

            def normalize(ext2, qc):
                # 1/sums for both heads: ln+exp on the partition-64 sums row
                # (same ACT table as the scores exp -> no table reload)
                rcpf = rcppool.tile([65, 2, QW], F32, tag="rcpf", name="rcpf")
                nc.scalar.activation(
                    rcpf[64:65, :, :], ext2[64:65, :, :], AF.Ln
                )
                rcpb = rcppool.tile([65, 2, QW], BF16, tag="rcpb", name="rcpb")
                nc.scalar.activation(
                    rcpb[64:65, :, :], rcpf[64:65, :, :], AF.Exp, scale=-1.0
                )
                for h in range(2):
                    rb = ps_m.tile([HS, QW], F32, tag="m", name="rb")
                    nc.tensor.matmul(
                        rb[:],
                        lhsT=ones64[64:65, :],
                        rhs=rcpb[64:65, h, :],
                        start=True,
                        stop=True,
                    )
                    nc.vector.tensor_tensor(
                        ocat[p][h * HS : (h + 1) * HS,
                                qc * QW : (qc + 1) * QW],
                        ext2[:HS, h, :],
                        rb[:],
                        ALU.mult,
                    )

            for qc in range(NTC):
                nkb = 4 * (qc + 1)
                for h in range(2):
                    ps_o_cur[h] = ps_av.tile([HS + 1, QW], F32, tag="av",
                                             name="ps_o")
                for kb in range(nkb):
                    ps_s = ps_qk.tile([P, 2, QW], F32, tag="qk", name="ps_s")
                    for h in range(2):
                        nc.tensor.matmul(
                            ps_s[:, h, :],
                            lhsT=rk[h * HS : (h + 1) * HS,
                                    kb * P : (kb + 1) * P],
                            rhs=rq[h * HS : (h + 1) * HS,
                                   qc * QW : (qc + 1) * QW],
                            start=True,
                            stop=True,
                        )
                    es = epool.tile([P, 2, QW], BF16, tag="es", name="es")
                    off = P * (kb - 4 * qc)
                    if off < 0:
                        nc.scalar.activation(
                            es[:].rearrange("p g q -> p (g q)"),
                            ps_s[:].rearrange("p g q -> p (g q)"),
                            AF.Exp,
                            scale=0.125,
                        )
                    else:  # diagonal block: partial exp + staircase mask
                        if off > 0:
                            nc.gpsimd.memset(es[:, :, 0:off], 0.0)
                        nc.scalar.activation(
                            es[:, :, off:], ps_s[:, :, off:],
                            AF.Exp, scale=0.125,
                        )
                        nc.gpsimd.tensor_tensor(
                            es[:, :, off : off + P],
                            es[:, :, off : off + P],
                            band_sb[:],
                            ALU.mult,
                        )
                    # Late priority: when several PE instructions are ready
                    # the scheduler prefers QKT/prep work, so AVs trail exp
                    # by as much as the es pool depth allows (robust to
                    # cost-model vs hardware timing skew).
                    with tc.high_priority(offset=-1_000_000):
                        for h in range(2):
                            nc.tensor.matmul(
                                ps_o_cur[h][:],
                                lhsT=vones[h][:, kb, :],
                                rhs=es[:, h, :],
                                start=(kb == 0),
                                stop=(kb == nkb - 1),
                            )
                    yield
                # pair 3's normalize feeds the out-projection: keep it eager
                # there; defer it elsewhere (nothing reads ocat until pair 3)
                norm_off = -1_000_000 if p < NPAIR - 1 else 0
                with tc.high_priority(offset=norm_off):
                    ext2 = extpool.tile([HS + 1, 2, QW], F32, tag="ext",
                                        name="ext2")
                    for h in range(2):
                        nc.vector.tensor_copy(ext2[:, h, :], ps_o_cur[h][:])
                    normalize(ext2, qc)
                yield
                if p == NPAIR - 1:
                    with tc.high_priority(offset=-1_000_000):
                        for tb in range(4 * qc, 4 * qc + 4):
                            emit_outproj_block(tb)
                    yield

        def drive(a_gen, b_gen, ratio=2):
            done_a = a_gen is None
            done_b = b_gen is None
            while not (done_a and done_b):
                if not done_a:
                    for _ in range(ratio):
                        try:
                            next(a_gen)
                        except StopIteration:
                            done_a = True
                            break
                if not done_b:
                    try:
                        next(b_gen)
                    except StopIteration:
                        done_b = True

        for _ in prep_stream(0, w_sb=w0_sb):
            pass
        for p in range(NPAIR):
            drive(
                attn_stream(p),
                prep_stream(p + 1) if p + 1 < NPAIR else None,
                ratio=2,
            )
    return _legalize_waits(nc)


def _rope_tables():
    inv = 1.0 / (1000.0 ** (np.arange(0, HS, 2, dtype=np.float64) / HS))
    t = np.arange(T, dtype=np.float64)[:, None] * inv[None, :]
    sinT = np.sin(t).astype(np.float32).T  # [32, T]
    cosT = np.cos(t).astype(np.float32).T
    cosF = np.concatenate([cosT] * 4, 0)  # [128, T]
    # sign layout for multiply-BEFORE-swap: sq = q*sinF, swapped(sq) lands as
    # [-v*sin; +u*sin] in the [u; v] destination slots.
    sinF = np.concatenate([sinT, -sinT, sinT, -sinT], 0)
    return cosF, sinF


def _host_prep():
    cosF, sinF = _rope_tables()
    psw = np.zeros((P, P), np.float32)
    for hh in range(2):
        o = hh * HS
        psw[o : o + 32, o + 32 : o + 64] = np.eye(32)
        psw[o + 32 : o + 64, o : o + 32] = np.eye(32)
    # band[p, j] = 1 iff j >= p: causal triangle for the 128-wide diagonal
    # band, replicated for the two packed heads
    tri = np.tril(np.ones((P, P), np.float32)).T
    band = np.concatenate([tri, tri], axis=1)  # [P, 2P]
    return cosF, sinF, psw, band


def kernel(x, W_attn, b_attn, W_o, b_o, _trace=False, _tmpdir=None):
    x = np.asarray(x, np.float32)
    W_attn = np.asarray(W_attn, np.float32)
    b_attn = np.asarray(b_attn, np.float32)
    W_o = np.asarray(W_o, np.float32)
    b_o = np.asarray(b_o, np.float32)

    if "nc" not in _cache:
        _cache["nc"] = _build_nc()
    nc = _cache["nc"]

    cosF, sinF, psw, band = _host_prep()
    cosF_b, sinF_b = cosF.astype(NPBF), sinF.astype(NPBF)
    psw_b, band_b = psw.astype(NPBF), band.astype(NPBF)

    def head_rows(h):  # q-rows of head h, evens then odds
        base = h * HS
        return np.concatenate(
            [np.arange(base, base + HS, 2), np.arange(base + 1, base + HS, 2)]
        )

    in_maps = []
    for core in range(NCORES):
        b, hg = core // 2, core % 2
        heads = [hg * 8 + i for i in range(8)]
        rows = []
        for p in range(NPAIR):
            h0, h1 = heads[2 * p], heads[2 * p + 1]
            qrows = np.concatenate([head_rows(h0), head_rows(h1)])
            rows += [qrows, C + qrows,
                     2 * C + np.concatenate([np.arange(h0 * HS, (h0 + 1) * HS),
                                             np.arange(h1 * HS, (h1 + 1) * HS)])]
        rows = np.concatenate(rows)  # [1536] in pair-major (q,k,v) order
        wqkvT = np.ascontiguousarray(W_attn[rows].T).astype(NPBF)  # [C, 1536]
        bqkv = np.ascontiguousarray(b_attn[rows].reshape(12, P))
        woT = np.ascontiguousarray(
            W_o[:, hg * 512 : (hg + 1) * 512].T
        ).astype(NPBF)  # [512, C]
        xT = np.ascontiguousarray(x[b].T).astype(NPBF)  # [C, T]
        in_maps.append(
            dict(xT=xT, wqkvT=wqkvT, bqkv=bqkv, woT=woT, cosF=cosF_b,
                 sinF=sinF_b, psw=psw_b, band=band_b)
        )

    res = run_bass_kernel_spmd(nc, in_maps, core_ids=list(range(NCORES)),
                               trace=_trace, tmpdir=_tmpdir)
    y = np.zeros((B, T, C), np.float32)
    for core in range(NCORES):
        y[core // 2] += res.results[core]["y"]
    y += b_o[None, None, :]
    if _trace:
        _cache["last_result"] = res
    return y
